# revision 1
# baseline (speedup 1.0000x reference)
"""Baichuan sliding-window GQA attention block on 8 trn2 NeuronCores.

Sharding: data-parallel over batch (2) x tensor-parallel over heads (4).
Core c handles batch b=c//4, head group g=c%4 (q heads 4g..4g+3, kv heads
2g..2g+1). Each core computes qkv projection, RoPE, 2-tap causal conv,
windowed attention and a row-sharded o_proj partial; the host sums the 4
partials per batch.

All on-chip tensors live in a transposed (feature, token) layout so the
tensor engine's contraction (partition) axis lines up without transposes:
  scoresT[k,q] = sum_d kT[d,k] qT[d,q];  outT[d,q] = sum_k v[k,d] probsT[k,q]
V alone is flipped to (token, dim) via PE transposes. Matmuls run as
float32r (full PE rate for moving dim >= 256, fp32 storage).
"""

import numpy as np
import ml_dtypes

B, S, H = 2, 2048, 2048
NH, NKV, HD = 16, 8, 128
WINDOW = 1024
THETA = 100000.0
TP = 4                      # tensor-parallel ways (head groups)
QH = NH // TP               # 4 q heads per core
KVH = NKV // TP             # 2 kv heads per core
NCORES = 8
SCALE = 1.0 / float(np.sqrt(HD))
NEG = -1.0e30

_CACHE = {}


def _build_program():
    import concourse.bacc as bacc
    import concourse.mybir as mybir
    import concourse.tile as tile

    f32 = mybir.dt.float32
    f32r = mybir.dt.float32r
    bf16 = mybir.dt.bfloat16
    Exp = mybir.ActivationFunctionType.Exp
    mult = mybir.AluOpType.mult
    add = mybir.AluOpType.add

    nc = bacc.Bacc("TRN2", target_bir_lowering=False, debug=False,
                   enable_asserts=False, num_devices=NCORES)

    hT_d = nc.dram_tensor("hT", [H, S], f32r, kind="ExternalInput")
    wpk_d = nc.dram_tensor("wpk", [H, 1024], f32r, kind="ExternalInput")
    wo_d = nc.dram_tensor("wo", [QH * HD, H], f32r, kind="ExternalInput")
    cs_d = nc.dram_tensor("cs", [128, S], f32, kind="ExternalInput")
    sn_d = nc.dram_tensor("sn", [128, S], f32, kind="ExternalInput")
    cw_d = nc.dram_tensor("cw", [128, 8], f32, kind="ExternalInput")
    msk_d = nc.dram_tensor("msk", [128, 2048], f32, kind="ExternalInput")
    eye_d = nc.dram_tensor("eye", [128, 128], f32, kind="ExternalInput")
    one_d = nc.dram_tensor("one", [128, 128], f32r, kind="ExternalInput")
    yT_d = nc.dram_tensor("yT", [H, S], f32, kind="ExternalOutput")

    NT = S // 256            # 8 token chunks of 256
    NK = H // 128            # 16 contraction tiles

    with tile.TileContext(nc) as tc:
        with (
            tc.tile_pool(name="const", bufs=1) as cp,
            tc.tile_pool(name="persist", bufs=1) as pp,
        ):
            cs_sb = cp.tile([128, S], f32, tag="cs", name="cs")
            sn_sb = cp.tile([128, S], f32, tag="sn", name="sn")
            cw_sb = cp.tile([128, 8], f32, tag="cw", name="cw")
            eye_sb = cp.tile([128, 128], f32, tag="eye", name="eye")
            one_sb = cp.tile([128, 128], f32r, tag="one", name="one")
            msk_sb = cp.tile([128, 2048], f32, tag="msk", name="msk")
            nc.sync.dma_start(out=cs_sb[:], in_=cs_d[:, :])
            nc.sync.dma_start(out=sn_sb[:], in_=sn_d[:, :])
            nc.sync.dma_start(out=cw_sb[:], in_=cw_d[:, :])
            nc.sync.dma_start(out=eye_sb[:], in_=eye_d[:, :])
            nc.sync.dma_start(out=one_sb[:], in_=one_d[:, :])

            # persistent across phases
            qpair = [pp.tile([128, 2 * S], f32r, tag=f"qp{i}", name=f"qp{i}") for i in range(KVH)]
            kconv = [pp.tile([128, S], f32r, tag=f"kc{i}", name=f"kc{i}") for i in range(KVH)]
            vt = [[pp.tile([128, 128], f32r, tag=f"vt{i}_{j}", name=f"vt{i}_{j}") for j in range(NK)]
                  for i in range(KVH)]

            # ---- phase B: fused qkv projection + rope + conv + v-transpose,
            # one f32r pass over hT in 256-token chunks. k/v staged in 2-chunk
            # rolling buffers; conv and the v transpose run per chunk.
            with (
                tc.tile_pool(name="bw", bufs=1) as bw,
                tc.tile_pool(name="bht", bufs=2) as bht,
                tc.tile_pool(name="broll", bufs=1) as br,
                tc.tile_pool(name="btmp", bufs=2) as bt,
                tc.tile_pool(name="bps", bufs=6, space="PSUM") as psb,
                tc.tile_pool(name="bps2", bufs=2, space="PSUM") as pse2,
                tc.tile_pool(name="bpst", bufs=1, space="PSUM") as pst,
            ):
                wf = [bw.tile([128, 1024], f32r, tag=f"wf{k}", name=f"wf{k}")
                      for k in range(NK)]
                hts0 = []
                for k in range(NK):
                    ht = bht.tile([128, 256], f32r, tag=f"ht{k}", name=f"ht{k}")
                    nc.sync.dma_start(out=ht[:],
                                      in_=hT_d[k * 128:(k + 1) * 128, 0:256])
                    hts0.append(ht)
                for k in range(NK):
                    nc.sync.dma_start(out=wf[k][:],
                                      in_=wpk_d[k * 128:(k + 1) * 128, :])
                kbuf = [br.tile([128, 512], f32, name=f"kbuf{i}") for i in range(KVH)]
                vbuf = [br.tile([128, 512], f32, name=f"vbuf{i}") for i in range(KVH)]
                for t in range(NT):
                    cur, prv = (t % 2) * 256, ((t + 1) % 2) * 256
                    if t == 0:
                        hts = hts0
                    else:
                        hts = []
                        for k in range(NK):
                            ht = bht.tile([128, 256], f32r, tag=f"ht{k}",
                                          name=f"ht{k}")
                            nc.sync.dma_start(
                                out=ht[:],
                                in_=hT_d[k * 128:(k + 1) * 128,
                                         t * 256:(t + 1) * 256])
                            hts.append(ht)
                    csl = cs_sb[:, t * 256:(t + 1) * 256]
                    snl = sn_sb[:, t * 256:(t + 1) * 256]
                    if t == 0:
                        # k-outer for the first chunk: 6 open accumulations so
                        # PE advances with each arriving weight tile instead of
                        # gating on the last one
                        psc0 = [psb.tile([128, 256], f32, tag="qkps",
                                         name=f"qk0_{c}") for c in range(6)]
                        for k in range(NK):
                            for c in range(6):
                                nc.tensor.matmul(
                                    psc0[c][:], wf[k][:, c * 128:(c + 1) * 128],
                                    hts[k][:],
                                    start=(k == 0), stop=(k == NK - 1))
                    for col in range(8):
                        if t == 0 and col < 6:
                            ps = psc0[col]
                        else:
                            ps = psb.tile([128, 256], f32, tag="qkps", name="qkps")
                            for k in range(NK):
                                nc.tensor.matmul(ps[:],
                                                 wf[k][:, col * 128:(col + 1) * 128],
                                                 hts[k][:],
                                                 start=(k == 0), stop=(k == NK - 1))
                        if col < 6:
                            e1 = bt.tile([128, 256], f32, tag="e1", name="e1")
                            e2 = pse2.tile([128, 256], f32, tag="e2", name="e2",
                                           bufs=1)
                            nc.vector.tensor_mul(e1[:], ps[:], csl)
                            nc.vector.tensor_mul(e2[:], ps[:], snl)
                            if col < 4:
                                dest = qpair[col // 2]
                                off = (col % 2) * S + t * 256
                            else:
                                dest = kbuf[col - 4]
                                off = cur
                            nc.vector.tensor_sub(dest[0:64, off:off + 256],
                                                 e1[0:64, :], e2[64:128, :])
                            nc.vector.tensor_add(dest[64:128, off:off + 256],
                                                 e2[0:64, :], e1[64:128, :])
                        else:
                            nc.scalar.copy(vbuf[col - 6][:, cur:cur + 256], ps[:])
                    # per-chunk conv (k -> kconv tile, v -> vcb) + v transpose
                    for i in range(KVH):
                        w0k, w1k = cw_sb[:, 2 * i:2 * i + 1], cw_sb[:, 2 * i + 1:2 * i + 2]
                        w0v, w1v = (cw_sb[:, 4 + 2 * i:5 + 2 * i],
                                    cw_sb[:, 5 + 2 * i:6 + 2 * i])
                        kc = kconv[i]
                        tmp = bt.tile([128, 256], f32, tag="ct", name="ct")
                        nc.vector.tensor_scalar_mul(tmp[:], kbuf[i][:, cur:cur + 256],
                                                    w1k)
                        nc.vector.scalar_tensor_tensor(
                            kc[:, t * 256 + 1:t * 256 + 256],
                            kbuf[i][:, cur:cur + 255], w0k, tmp[:, 1:256], mult, add)
                        if t == 0:
                            nc.vector.tensor_copy(kc[:, 0:1], tmp[:, 0:1])
                        else:
                            nc.vector.scalar_tensor_tensor(
                                kc[:, t * 256:t * 256 + 1],
                                kbuf[i][:, prv + 255:prv + 256], w0k,
                                tmp[:, 0:1], mult, add)
                        vcb = bt.tile([128, 256], f32, tag="vcb", name="vcb")
                        tm2 = bt.tile([128, 256], f32, tag="ct2", name="ct2")
                        nc.vector.tensor_scalar_mul(tm2[:], vbuf[i][:, cur:cur + 256],
                                                    w1v)
                        nc.vector.scalar_tensor_tensor(
                            vcb[:, 1:256], vbuf[i][:, cur:cur + 255], w0v,
                            tm2[:, 1:256], mult, add)
                        if t == 0:
                            nc.vector.tensor_copy(vcb[:, 0:1], tm2[:, 0:1])
                        else:
                            nc.vector.scalar_tensor_tensor(
                                vcb[:, 0:1], vbuf[i][:, prv + 255:prv + 256], w0v,
                                tm2[:, 0:1], mult, add)
                        for h in range(2):
                            tp = pst.tile([128, 128], f32, tag="vtp", name="vtp")
                            nc.tensor.transpose(tp[:], vcb[:, h * 128:(h + 1) * 128],
                                                eye_sb[:])
                            nc.vector.tensor_copy(vt[i][2 * t + h][:], tp[:])

            nc.sync.dma_start(out=msk_sb[:], in_=msk_d[:, :])
            # ---- phase E: banded attention;  phase F: o_proj partial ----
            with (
                tc.tile_pool(name="att", bufs=1) as ap,
                tc.tile_pool(name="atw", bufs=3) as aw,
            ):
                wo_sb = [ap.tile([128, H], f32r, tag=f"wo{d}", name=f"wo{d}")
                         for d in range(QH)]
                for d in range(QH):
                    nc.sync.dma_start(out=wo_sb[d][:],
                                      in_=wo_d[d * 128:(d + 1) * 128, :])
                attn = [ap.tile([128, S], f32r, tag=f"at{h}", name=f"at{h}")
                        for h in range(QH)]

                with (
                    tc.tile_pool(name="eps_sc", bufs=3, space="PSUM") as pss,
                    tc.tile_pool(name="eps_pv", bufs=2, space="PSUM") as psv,
                    tc.tile_pool(name="eps_sm", bufs=1, space="PSUM") as psm,
                    tc.tile_pool(name="fps", bufs=2, space="PSUM") as psf,
                ):
                  for qi in range(NT):
                    for i in range(KVH):
                        qc = qi * 256
                        jstart = max(0, qc // 128 - 8)
                        jend = qc // 128 + 1
                        ps_o = psv.tile([128, 512], f32, tag="pv", name="pv")
                        ps_s = psm.tile([1, 512], f32, tag="sm", name="sm")
                        jorder = list(range(jstart, jend + 1))
                        if qc - jstart * 128 == 1024:
                            # window-edge tile is half-masked; run it second so
                            # a full tile carries the start=True psum clear
                            jorder[0], jorder[1] = jorder[1], jorder[0]
                        jfirst = jorder[0]
                        for j in jorder:
                            ps_sc = pss.tile([128, 512], f32, tag="sc", name="sc")
                            lhs = kconv[i][:, j * 128:(j + 1) * 128]
                            q2 = qpair[i][:].rearrange("p (h s) -> p h s", h=2)
                            nc.tensor.matmul(
                                ps_sc[:], lhs,
                                q2[:, :, qc:qc + 256],
                                start=True, stop=True)
                            mt = {1024: 0, 896: 1, 0: 2, -128: 3}.get(qc - j * 128)
                            pb = aw.tile([128, 512], f32r, tag="pb", name="pb",
                                         bufs=6)
                            if j != jfirst and qc - j * 128 == 1024:
                                # window-edge tile: second q-half of each head is
                                # fully masked -> valid halves only (never the
                                # start matmul, so the psum clear is full)
                                ps3 = ps_sc.rearrange("p (h q) -> p h q", h=2)
                                pb3 = pb.rearrange("p (h q) -> p h q", h=2)
                                po3 = ps_o.rearrange("p (h q) -> p h q", h=2)
                                pss3 = ps_s.rearrange("p (h q) -> p h q", h=2)
                                mk3 = msk_sb[:, 0:512].rearrange(
                                    "p (h q) -> p h q", h=2)
                                tm = aw.tile([128, 512], f32, tag="tm", name="tm")
                                tm3 = tm.rearrange("p (h q) -> p h q", h=2)
                                nc.vector.tensor_add(
                                    tm3[:, :, 0:128], ps3[:, :, 0:128],
                                    mk3[:, :, 0:128])
                                nc.scalar.activation(pb3[:, :, 0:128],
                                                     tm3[:, :, 0:128], Exp,
                                                     bias=0.0, scale=SCALE)
                                nc.tensor.matmul(po3[:, :, 0:128], vt[i][j][:],
                                                 pb3[:, :, 0:128],
                                                 start=False, stop=False)
                                nc.tensor.matmul(pss3[:, :, 0:128],
                                                 one_sb[:, 0:1],
                                                 pb3[:, :, 0:128],
                                                 start=False, stop=False)
                                continue
                            if j == jend:
                                # delta=-128 tile: first q-half of each head is
                                # fully masked -> process only the valid halves
                                # via a 2-segment AP (n=256 keeps f32r rate)
                                ps3 = ps_sc.rearrange("p (h q) -> p h q", h=2)
                                pb3 = pb.rearrange("p (h q) -> p h q", h=2)
                                po3 = ps_o.rearrange("p (h q) -> p h q", h=2)
                                pss3 = ps_s.rearrange("p (h q) -> p h q", h=2)
                                mk3 = msk_sb[:, mt * 512:(mt + 1) * 512].rearrange(
                                    "p (h q) -> p h q", h=2)
                                tm = aw.tile([128, 512], f32, tag="tm", name="tm")
                                tm3 = tm.rearrange("p (h q) -> p h q", h=2)
                                nc.vector.tensor_add(
                                    tm3[:, :, 128:256], ps3[:, :, 128:256],
                                    mk3[:, :, 128:256])
                                nc.scalar.activation(pb3[:, :, 128:256],
                                                     tm3[:, :, 128:256], Exp,
                                                     bias=0.0, scale=SCALE)
                                nc.tensor.matmul(po3[:, :, 128:256], vt[i][j][:],
                                                 pb3[:, :, 128:256],
                                                 start=False, stop=True)
                                nc.tensor.matmul(pss3[:, :, 128:256],
                                                 one_sb[:, 0:1],
                                                 pb3[:, :, 128:256],
                                                 start=False, stop=True)
                                continue
                            if mt is None:
                                nc.scalar.activation(pb[:], ps_sc[:], Exp,
                                                     bias=0.0, scale=SCALE)
                            else:
                                tm = aw.tile([128, 512], f32, tag="tm", name="tm")
                                nc.vector.tensor_add(
                                    tm[:], ps_sc[:],
                                    msk_sb[:, mt * 512:(mt + 1) * 512])
                                nc.scalar.activation(pb[:], tm[:], Exp,
                                                     bias=0.0, scale=SCALE)
                            nc.tensor.matmul(ps_o[:], vt[i][j][:], pb[:],
                                             start=(j == jfirst), stop=(j == jend))
                            nc.tensor.matmul(ps_s[:], one_sb[:, 0:1], pb[:],
                                             start=(j == jfirst), stop=(j == jend))
                        rsum = aw.tile([1, 512], f32, tag="rs", name="rs")
                        nc.vector.reciprocal(rsum[:], ps_s[:])
                        rb = aw.tile([128, 512], f32, tag="rb", name="rb")
                        nc.gpsimd.partition_broadcast(rb[:], rsum[:])
                        nc.vector.tensor_mul(attn[2 * i][:, qc:qc + 256],
                                             ps_o[:, 0:256], rb[:, 0:256])
                        nc.vector.tensor_mul(attn[2 * i + 1][:, qc:qc + 256],
                                             ps_o[:, 256:512], rb[:, 256:512])

                  for t4 in range(4):
                    for oc in range(NK):
                        ps_y = psf.tile([128, 512], f32, tag="y", name="y")
                        for d in range(QH):
                            nc.tensor.matmul(
                                ps_y[:],
                                wo_sb[d][:, oc * 128:(oc + 1) * 128],
                                attn[d][:, t4 * 512:(t4 + 1) * 512],
                                start=(d == 0), stop=(d == QH - 1))
                        yb = aw.tile([128, 512], f32, tag="yb", name="yb",
                                     bufs=4)
                        if (oc + t4) % 2 == 0:
                            nc.vector.tensor_copy(yb[:], ps_y[:])
                        else:
                            nc.scalar.copy(yb[:], ps_y[:])
                        nc.sync.dma_start(
                            out=yT_d[oc * 128:(oc + 1) * 128,
                                     t4 * 512:(t4 + 1) * 512],
                            in_=yb[:])

    nc.finalize()
    return nc


def _host_inputs(hidden, W_pack, W_o, conv_k, conv_v):
    """Per-core input maps."""
    pos = np.arange(S, dtype=np.float64)
    inv_freq = 1.0 / (THETA ** (np.arange(0, HD, 2, dtype=np.float64) / HD))
    freqs = np.outer(pos, inv_freq)                       # (S, 64)
    cos = np.cos(freqs).T.astype(np.float32)              # (64, S)
    sin = np.sin(freqs).T.astype(np.float32)
    cs = np.concatenate([cos, cos], axis=0)               # (128, S)
    sn = np.concatenate([sin, sin], axis=0)

    kk = np.arange(128)[:, None]
    qq = np.arange(256)[None, :]
    def double(m):
        return np.concatenate([m, m], axis=1).astype(np.float32)
    t0 = double(np.where(kk <= qq, 0.0, NEG))             # delta = 0
    tm128 = double(np.where(kk <= qq - 128, 0.0, NEG))    # delta = -128
    w896 = double(np.where(qq - kk < 128, 0.0, NEG))      # delta = 896
    w1024 = double(np.where(qq < kk, 0.0, NEG))           # delta = 1024
    msk = np.concatenate([w1024, w896, t0, tm128], axis=1)  # (128, 2048)

    eye = np.eye(128, dtype=np.float32)
    one = np.ones((128, 128), dtype=np.float32)

    in_maps = []
    for c in range(NCORES):
        b, g = c // TP, c % TP
        hT = np.ascontiguousarray(hidden[b].T).astype(np.float32)
        wq = W_pack[:, g * 512:(g + 1) * 512]
        wk = W_pack[:, NH * HD + 2 * g * 128: NH * HD + (2 * g + 2) * 128]
        wv = W_pack[:, NH * HD + NKV * HD + 2 * g * 128:
                    NH * HD + NKV * HD + (2 * g + 2) * 128]
        wpk = np.ascontiguousarray(
            np.concatenate([wq, wk, wv], axis=1)).astype(np.float32)
        wo = np.ascontiguousarray(
            W_o[g * 512:(g + 1) * 512, :]).astype(np.float32)
        cwv = np.empty(8, np.float32)
        for i in range(KVH):
            cwv[2 * i] = conv_k[2 * g + i, 0]
            cwv[2 * i + 1] = conv_k[2 * g + i, 1]
            cwv[4 + 2 * i] = conv_v[2 * g + i, 0]
            cwv[4 + 2 * i + 1] = conv_v[2 * g + i, 1]
        cw = np.broadcast_to(cwv, (128, 8)).copy()
        in_maps.append({
            "hT": hT, "wpk": wpk, "wo": wo, "cs": cs, "sn": sn,
            "cw": cw, "msk": msk, "eye": eye, "one": one,
        })
    return in_maps


def run_cores(in_maps, trace=False, **kw):
    from concourse.bass_utils import run_bass_kernel_spmd
    if "nc" not in _CACHE:
        _CACHE["nc"] = _build_program()
    return run_bass_kernel_spmd(_CACHE["nc"], in_maps, list(range(NCORES)),
                                trace=trace, **kw)


def kernel(hidden, W_pack, W_o, conv_k, conv_v):
    hidden = np.asarray(hidden, np.float32)
    W_pack = np.asarray(W_pack, np.float32)
    W_o = np.asarray(W_o, np.float32)
    conv_k = np.asarray(conv_k, np.float32)
    conv_v = np.asarray(conv_v, np.float32)
    in_maps = _host_inputs(hidden, W_pack, W_o, conv_k, conv_v)
    res = run_cores(in_maps)
    out = np.zeros((B, S, H), np.float32)
    for c in range(NCORES):
        b = c // TP
        out[b] += res.results[c]["yT"].T
    return out



# revision 17
# speedup vs baseline: 1.0659x; 1.0659x over previous
"""Baichuan sliding-window GQA attention block on 8 trn2 NeuronCores.

Sharding: data-parallel over batch (2) x tensor-parallel over heads (4).
Core c handles batch b=c//4, head group g=c%4 (q heads 4g..4g+3, kv heads
2g..2g+1). Each core computes qkv projection, RoPE, 2-tap causal conv,
windowed attention and a row-sharded o_proj partial; the host sums the 4
partials per batch.

All on-chip tensors live in a transposed (feature, token) layout so the
tensor engine's contraction (partition) axis lines up without transposes:
  scoresT[k,q] = sum_d kT[d,k] qT[d,q];  outT[d,q] = sum_k v[k,d] probsT[k,q]
V alone is flipped to (token, dim) via PE transposes.

Matmul inputs (hidden, W_pack, W_o, cos/sin) are staged in bf16: same PE
rate as f32r but half the DMA bytes. The first chunk's weights and
activations are packed into one interleaved "stage0" blob so the PE can
start after the first ~3us DMA group instead of waiting for all weights.
Attention internals (scores, probs, V) stay fp32.
"""

import numpy as np
import ml_dtypes

B, S, H = 2, 2048, 2048
NH, NKV, HD = 16, 8, 128
WINDOW = 1024
THETA = 100000.0
TP = 4                      # tensor-parallel ways (head groups)
QH = NH // TP               # 4 q heads per core
KVH = NKV // TP             # 2 kv heads per core
NCORES = 8
SCALE = 1.0 / float(np.sqrt(HD))
NEG = -1.0e30

NT = S // 256               # 8 token chunks of 256
NK = H // 128               # 16 contraction tiles

# stage0 blob layout (bf16 cols): prefix then 8 groups of (wf|h0)x2
_PRE = [("cs0", 256), ("sn0", 256)]
_PREN = sum(n for _, n in _PRE)          # 512
_GRP = 2 * (1024 + 256)                  # 2560
_ST0 = _PREN + 8 * _GRP                  # 21248

_CACHE = {}


def _build_program():
    import concourse.bacc as bacc
    import concourse.mybir as mybir
    import concourse.tile as tile

    f32 = mybir.dt.float32
    f32r = mybir.dt.float32r
    bf16 = mybir.dt.bfloat16
    Exp = mybir.ActivationFunctionType.Exp
    mult = mybir.AluOpType.mult
    add = mybir.AluOpType.add

    nc = bacc.Bacc("TRN2", target_bir_lowering=False, debug=False,
                   enable_asserts=False, num_devices=NCORES)

    cw_d = nc.dram_tensor("cw", [128, 8], f32, kind="ExternalInput")
    oe_d = nc.dram_tensor("oe", [128, 256], f32r, kind="ExternalInput")
    st0_d = nc.dram_tensor("st0", [128, _ST0], bf16, kind="ExternalInput")
    csn_d = nc.dram_tensor("csn", [128, 2 * S], bf16, kind="ExternalInput")
    hb_d = nc.dram_tensor("hb", [128, 7 * 4096], bf16, kind="ExternalInput")
    wob_d = nc.dram_tensor("wob", [128, QH * 2048], bf16, kind="ExternalInput")
    msk_d = nc.dram_tensor("msk", [128, 2048], f32, kind="ExternalInput")
    yT_d = nc.dram_tensor("yT", [H, S], f32, kind="ExternalOutput")

    with tile.TileContext(nc) as tc:
        with (
            tc.tile_pool(name="const", bufs=1) as cp,
            tc.tile_pool(name="persist", bufs=1) as pp,
        ):
            cw_sb = cp.tile([128, 8], f32, tag="cw", name="cw_sb")
            oe_sb = cp.tile([128, 256], f32r, tag="oe", name="oe_sb")
            wo_sb = cp.tile([128, QH * 2048], bf16, tag="wob", name="wo_sb")
            msk_sb = cp.tile([128, 2048], f32, tag="msk", name="msk_sb")

            # persistent across phases
            qpair = [pp.tile([128, 2 * S], f32r, tag=f"qp{i}", name=f"qp{i}") for i in range(KVH)]
            kconv = [pp.tile([128, S], f32r, tag=f"kc{i}", name=f"kc{i}") for i in range(KVH)]
            vt = [[pp.tile([128, 128], f32r, tag=f"vt{i}_{j}", name=f"vt{i}_{j}") for j in range(NK)]
                  for i in range(KVH)]

            with tc.tile_pool(name="bst", bufs=1) as bs:
                st0 = bs.tile([128, _ST0], bf16, tag="st0", name="st0")
                # views into the stage0 blob
                off = {}
                a = 0
                for nm, n in _PRE:
                    off[nm] = a
                    a += n
                one_sb = oe_sb[:, 0:128]
                eye_sb = oe_sb[:, 128:256]

                def wfv(k):
                    base = _PREN + (k // 2) * _GRP + (k % 2) * 1280
                    return st0[:, base:base + 1024]

                def h0v(k):
                    base = _PREN + (k // 2) * _GRP + (k % 2) * 1280 + 1024
                    return st0[:, base:base + 256]

                # ---- phase B: fused qkv projection + rope + conv +
                # v-transpose. stage0 brings the weights interleaved with
                # chunk-0 tokens in 8 DMA groups; chunks 1-7 stream as one
                # blob DMA each.
                with (
                    tc.tile_pool(name="bcsn", bufs=1) as bc,
                    tc.tile_pool(name="bhb", bufs=3) as bh,
                    tc.tile_pool(name="broll", bufs=1) as br,
                    tc.tile_pool(name="btmp", bufs=2) as bt,
                    tc.tile_pool(name="bps", bufs=4, space="PSUM") as psb,
                    tc.tile_pool(name="bps2", bufs=2, space="PSUM") as pse2,
                    tc.tile_pool(name="bpst", bufs=1, space="PSUM") as pst,
                ):
                    # issue order on SP = DMA service order: cw, stage0
                    # groups (prefix rides with group 0), then chunk 1,
                    # cos/sin, chunk 2, wo, msk, chunks 3-7 (their tile-reuse
                    # waits park SP, which is idle until the output writes).
                    nc.sync.dma_start(out=cw_sb[:], in_=cw_d[:, :])
                    nc.sync.dma_start(out=oe_sb[:], in_=oe_d[:, :])
                    nc.sync.dma_start(out=st0[:, 0:_PREN + _GRP],
                                      in_=st0_d[:, 0:_PREN + _GRP])
                    for g in range(1, 8):
                        ga = _PREN + g * _GRP
                        nc.sync.dma_start(out=st0[:, ga:ga + _GRP],
                                          in_=st0_d[:, ga:ga + _GRP])
                    csn_sb = bc.tile([128, 2 * S], bf16, tag="csn", name="csn_sb")
                    hbt = []
                    for t in range(1, NT):
                        ht = bh.tile([128, 4096], bf16, tag="hb", name=f"hb{t}")
                        hbt.append(ht)
                        nc.sync.dma_start(out=ht[:],
                                          in_=hb_d[:, (t - 1) * 4096:t * 4096])
                        if t == 1:
                            nc.sync.dma_start(out=csn_sb[:], in_=csn_d[:, :])
                        elif t == 2:
                            nc.sync.dma_start(out=wo_sb[:], in_=wob_d[:, :])
                            nc.sync.dma_start(out=msk_sb[:], in_=msk_d[:, :])

                    kbuf = [br.tile([128, 512], f32, name=f"kbuf{i}") for i in range(KVH)]
                    vbuf = [br.tile([128, 512], f32, name=f"vbuf{i}") for i in range(KVH)]
                    for t in range(NT):
                        cur, prv = (t % 2) * 256, ((t + 1) % 2) * 256
                        if t == 0:
                            csl = st0[:, off["cs0"]:off["cs0"] + 256]
                            snl = st0[:, off["sn0"]:off["sn0"] + 256]
                        else:
                            csl = csn_sb[:, t * 256:(t + 1) * 256]
                            snl = csn_sb[:, S + t * 256:S + (t + 1) * 256]
                        if t == 0:
                            # k-outer over all 8 cols (4 psum banks of col
                            # pairs): each arriving (wf,h0) DMA group unlocks
                            # 16 matmuls, so PE ramps with the DMA
                            psc0 = [psb.tile([128, 512], f32, tag="qkps",
                                             name=f"qk0_{c}") for c in range(4)]
                            for k in range(NK):
                                for c in range(8):
                                    # start=True lazily zeroes the whole 2KB
                                    # psum bank: only the even half may carry
                                    # it; the odd half's k=0 lands on bytes
                                    # already marked pending-zero and
                                    # write-clears them
                                    nc.tensor.matmul(
                                        psc0[c // 2][:, (c % 2) * 256:(c % 2) * 256 + 256],
                                        wfv(k)[:, c * 128:(c + 1) * 128],
                                        h0v(k),
                                        start=(k == 0 and c % 2 == 0),
                                        stop=(k == NK - 1),
                                        skip_group_check=True)
                        for c4 in range(4):
                            if t == 0:
                                ps2 = psc0[c4]
                            else:
                                ps2 = psb.tile([128, 512], f32, tag="qkps",
                                               name="qkps")
                                for hh in range(2):
                                    col = 2 * c4 + hh
                                    for k in range(NK):
                                        nc.tensor.matmul(
                                            ps2[:, hh * 256:(hh + 1) * 256],
                                            wfv(k)[:, col * 128:(col + 1) * 128],
                                            hbt[t - 1][:, k * 256:(k + 1) * 256],
                                            start=(k == 0), stop=(k == NK - 1))
                            for hh in range(2):
                                col = 2 * c4 + hh
                                ps = ps2[:, hh * 256:(hh + 1) * 256]
                                if col < 6:
                                    e1 = bt.tile([128, 256], f32, tag="e1", name="e1")
                                    e2 = pse2.tile([128, 256], f32, tag="e2",
                                                   name="e2", bufs=1)
                                    nc.vector.tensor_mul(e1[:], ps, csl)
                                    nc.vector.tensor_mul(e2[:], ps, snl)
                                    if col < 4:
                                        dest = qpair[col // 2]
                                        doff = (col % 2) * S + t * 256
                                    else:
                                        dest = kbuf[col - 4]
                                        doff = cur
                                    nc.vector.tensor_sub(dest[0:64, doff:doff + 256],
                                                         e1[0:64, :], e2[64:128, :])
                                    nc.vector.tensor_add(dest[64:128, doff:doff + 256],
                                                         e2[0:64, :], e1[64:128, :])
                                else:
                                    nc.scalar.copy(vbuf[col - 6][:, cur:cur + 256],
                                                   ps)
                        # per-chunk conv (k -> kconv, v -> vcb) + v transpose
                        for i in range(KVH):
                            w0k, w1k = cw_sb[:, 2 * i:2 * i + 1], cw_sb[:, 2 * i + 1:2 * i + 2]
                            w0v, w1v = (cw_sb[:, 4 + 2 * i:5 + 2 * i],
                                        cw_sb[:, 5 + 2 * i:6 + 2 * i])
                            kc = kconv[i]
                            tmp = bt.tile([128, 256], f32, tag="ct", name="ct")
                            nc.vector.tensor_scalar_mul(tmp[:], kbuf[i][:, cur:cur + 256],
                                                        w1k)
                            nc.vector.scalar_tensor_tensor(
                                kc[:, t * 256 + 1:t * 256 + 256],
                                kbuf[i][:, cur:cur + 255], w0k, tmp[:, 1:256], mult, add)
                            if t == 0:
                                nc.vector.tensor_copy(kc[:, 0:1], tmp[:, 0:1])
                            else:
                                nc.vector.scalar_tensor_tensor(
                                    kc[:, t * 256:t * 256 + 1],
                                    kbuf[i][:, prv + 255:prv + 256], w0k,
                                    tmp[:, 0:1], mult, add)
                            vcb = bt.tile([128, 256], f32r, tag="vcb", name="vcb")
                            tm2 = bt.tile([128, 256], f32, tag="ct2", name="ct2")
                            nc.vector.tensor_scalar_mul(tm2[:], vbuf[i][:, cur:cur + 256],
                                                        w1v)
                            nc.vector.scalar_tensor_tensor(
                                vcb[:, 1:256], vbuf[i][:, cur:cur + 255], w0v,
                                tm2[:, 1:256], mult, add)
                            if t == 0:
                                nc.vector.tensor_copy(vcb[:, 0:1], tm2[:, 0:1])
                            else:
                                nc.vector.scalar_tensor_tensor(
                                    vcb[:, 0:1], vbuf[i][:, prv + 255:prv + 256], w0v,
                                    tm2[:, 0:1], mult, add)
                            for h in range(2):
                                tp = pst.tile([128, 128], f32r, tag="vtp", name="vtp")
                                nc.tensor.transpose(tp[:], vcb[:, h * 128:(h + 1) * 128],
                                                    eye_sb)
                                nc.scalar.copy(vt[i][2 * t + h][:], tp[:])

                # ---- phase E: banded attention; phase F: o_proj partial ----
                with (
                    tc.tile_pool(name="att", bufs=1) as ap,
                    tc.tile_pool(name="atw", bufs=3) as aw,
                    tc.tile_pool(name="eps_sc", bufs=3, space="PSUM") as pss,
                    tc.tile_pool(name="eps_pv", bufs=2, space="PSUM") as psv,
                    tc.tile_pool(name="eps_sm", bufs=1, space="PSUM") as psm,
                    tc.tile_pool(name="fps", bufs=2, space="PSUM") as psf,
                ):
                  attn = [ap.tile([128, S], bf16, tag=f"at{h}", name=f"at{h}")
                          for h in range(QH)]
                  for qi in range(NT):
                    for i in range(KVH):
                        qc = qi * 256
                        jstart = max(0, qc // 128 - 8)
                        jend = qc // 128 + 1
                        ps_o = psv.tile([128, 512], f32, tag="pv", name="pv")
                        ps_s = psm.tile([1, 512], f32, tag="sm", name="sm")
                        jorder = list(range(jstart, jend + 1))
                        if qc - jstart * 128 == 1024:
                            # window-edge tile is half-masked; run it second so
                            # a full tile carries the start=True psum clear
                            jorder[0], jorder[1] = jorder[1], jorder[0]
                        jfirst = jorder[0]
                        for j in jorder:
                            ps_sc = pss.tile([128, 512], f32, tag="sc", name="sc")
                            lhs = kconv[i][:, j * 128:(j + 1) * 128]
                            q2 = qpair[i][:].rearrange("p (h s) -> p h s", h=2)
                            mt = {1024: 0, 896: 1, 0: 2, -128: 3}.get(qc - j * 128)
                            pb = aw.tile([128, 512], f32r, tag="pb", name="pb",
                                         bufs=6)
                            if j != jfirst and qc - j * 128 == 1024:
                                # window-edge tile: second q-half of each head
                                # is fully masked -> valid halves only (never
                                # the start matmul, so the psum clear is full)
                                ps3 = ps_sc.rearrange("p (h q) -> p h q", h=2)
                                nc.tensor.matmul(
                                    ps3[:, :, 0:128], lhs,
                                    q2[:, :, qc:qc + 128],
                                    start=True, stop=True)
                                pb3 = pb.rearrange("p (h q) -> p h q", h=2)
                                po3 = ps_o.rearrange("p (h q) -> p h q", h=2)
                                pss3 = ps_s.rearrange("p (h q) -> p h q", h=2)
                                mk3 = msk_sb[:, 0:512].rearrange(
                                    "p (h q) -> p h q", h=2)
                                tm = aw.tile([128, 512], f32, tag="tm", name="tm")
                                tm3 = tm.rearrange("p (h q) -> p h q", h=2)
                                nc.vector.tensor_add(
                                    tm3[:, :, 0:128], ps3[:, :, 0:128],
                                    mk3[:, :, 0:128])
                                nc.scalar.activation(pb3[:, :, 0:128],
                                                     tm3[:, :, 0:128], Exp,
                                                     bias=0.0, scale=SCALE)
                                nc.tensor.matmul(po3[:, :, 0:128], vt[i][j][:],
                                                 pb3[:, :, 0:128],
                                                 start=False, stop=False)
                                nc.tensor.matmul(pss3[:, :, 0:128],
                                                 one_sb[:, 0:1],
                                                 pb3[:, :, 0:128],
                                                 start=False, stop=False)
                                continue
                            if j == jend:
                                # delta=-128 tile: first q-half of each head is
                                # fully masked -> process only the valid halves
                                # via a 2-segment AP (n=256 keeps f32r rate)
                                ps3 = ps_sc.rearrange("p (h q) -> p h q", h=2)
                                nc.tensor.matmul(
                                    ps3[:, :, 128:256], lhs,
                                    q2[:, :, qc + 128:qc + 256],
                                    start=True, stop=True)
                                pb3 = pb.rearrange("p (h q) -> p h q", h=2)
                                po3 = ps_o.rearrange("p (h q) -> p h q", h=2)
                                pss3 = ps_s.rearrange("p (h q) -> p h q", h=2)
                                mk3 = msk_sb[:, mt * 512:(mt + 1) * 512].rearrange(
                                    "p (h q) -> p h q", h=2)
                                tm = aw.tile([128, 512], f32, tag="tm", name="tm")
                                tm3 = tm.rearrange("p (h q) -> p h q", h=2)
                                nc.vector.tensor_add(
                                    tm3[:, :, 128:256], ps3[:, :, 128:256],
                                    mk3[:, :, 128:256])
                                nc.scalar.activation(pb3[:, :, 128:256],
                                                     tm3[:, :, 128:256], Exp,
                                                     bias=0.0, scale=SCALE)
                                nc.tensor.matmul(po3[:, :, 128:256], vt[i][j][:],
                                                 pb3[:, :, 128:256],
                                                 start=False, stop=True)
                                nc.tensor.matmul(pss3[:, :, 128:256],
                                                 one_sb[:, 0:1],
                                                 pb3[:, :, 128:256],
                                                 start=False, stop=True)
                                continue
                            nc.tensor.matmul(
                                ps_sc[:], lhs,
                                q2[:, :, qc:qc + 256],
                                start=True, stop=True)
                            if mt is None:
                                nc.scalar.activation(pb[:], ps_sc[:], Exp,
                                                     bias=0.0, scale=SCALE)
                            else:
                                tm = aw.tile([128, 512], f32, tag="tm", name="tm")
                                nc.vector.tensor_add(
                                    tm[:], ps_sc[:],
                                    msk_sb[:, mt * 512:(mt + 1) * 512])
                                nc.scalar.activation(pb[:], tm[:], Exp,
                                                     bias=0.0, scale=SCALE)
                            nc.tensor.matmul(ps_o[:], vt[i][j][:], pb[:],
                                             start=(j == jfirst), stop=(j == jend))
                            nc.tensor.matmul(ps_s[:], one_sb[:, 0:1], pb[:],
                                             start=(j == jfirst), stop=(j == jend))
                        rsum = aw.tile([1, 512], f32, tag="rs", name="rs")
                        nc.vector.reciprocal(rsum[:], ps_s[:])
                        rb = aw.tile([128, 512], f32, tag="rb", name="rb")
                        nc.gpsimd.partition_broadcast(rb[:], rsum[:])
                        nc.vector.tensor_mul(attn[2 * i][:, qc:qc + 256],
                                             ps_o[:, 0:256], rb[:, 0:256])
                        nc.vector.tensor_mul(attn[2 * i + 1][:, qc:qc + 256],
                                             ps_o[:, 256:512], rb[:, 256:512])

                  for t4 in range(4):
                    for oc in range(NK):
                        ps_y = psf.tile([128, 512], f32, tag="y", name="y")
                        for d in range(QH):
                            nc.tensor.matmul(
                                ps_y[:],
                                wo_sb[:, d * 2048 + oc * 128:d * 2048 + (oc + 1) * 128],
                                attn[d][:, t4 * 512:(t4 + 1) * 512],
                                start=(d == 0), stop=(d == QH - 1))
                        yb = aw.tile([128, 512], f32, tag="yb", name="yb",
                                     bufs=4)
                        if (oc + t4) % 2 == 0:
                            nc.vector.tensor_copy(yb[:], ps_y[:])
                        else:
                            nc.scalar.copy(yb[:], ps_y[:])
                        nc.sync.dma_start(
                            out=yT_d[oc * 128:(oc + 1) * 128,
                                     t4 * 512:(t4 + 1) * 512],
                            in_=yb[:])

    nc.finalize()
    return nc


def _host_inputs(hidden, W_pack, W_o, conv_k, conv_v):
    """Per-core input maps (bf16 staged)."""
    bf = ml_dtypes.bfloat16
    pos = np.arange(S, dtype=np.float64)
    inv_freq = 1.0 / (THETA ** (np.arange(0, HD, 2, dtype=np.float64) / HD))
    freqs = np.outer(pos, inv_freq)                       # (S, 64)
    cos = np.cos(freqs).T.astype(np.float32)              # (64, S)
    sin = np.sin(freqs).T.astype(np.float32)
    cs = np.concatenate([cos, cos], axis=0)               # (128, S)
    sn = np.concatenate([sin, sin], axis=0)

    kk = np.arange(128)[:, None]
    qq = np.arange(256)[None, :]
    def double(m):
        return np.concatenate([m, m], axis=1).astype(np.float32)
    t0 = double(np.where(kk <= qq, 0.0, NEG))             # delta = 0
    tm128 = double(np.where(kk <= qq - 128, 0.0, NEG))    # delta = -128
    w896 = double(np.where(qq - kk < 128, 0.0, NEG))      # delta = 896
    w1024 = double(np.where(qq < kk, 0.0, NEG))           # delta = 1024
    msk = np.concatenate([w1024, w896, t0, tm128], axis=1)  # (128, 2048)

    eye = np.eye(128, dtype=np.float32)

    csn = np.concatenate([cs, sn], axis=1).astype(bf)     # (128, 4096)

    in_maps = []
    for c in range(NCORES):
        b, g = c // TP, c % TP
        hT = np.ascontiguousarray(hidden[b].T).astype(bf)     # (2048, 2048)
        wq = W_pack[:, g * 512:(g + 1) * 512]
        wk = W_pack[:, NH * HD + 2 * g * 128: NH * HD + (2 * g + 2) * 128]
        wv = W_pack[:, NH * HD + NKV * HD + 2 * g * 128:
                    NH * HD + NKV * HD + (2 * g + 2) * 128]
        wpk = np.ascontiguousarray(
            np.concatenate([wq, wk, wv], axis=1)).astype(bf)  # (2048, 1024)
        cwv = np.empty(8, np.float32)
        for i in range(KVH):
            cwv[2 * i] = conv_k[2 * g + i, 0]
            cwv[2 * i + 1] = conv_k[2 * g + i, 1]
            cwv[4 + 2 * i] = conv_v[2 * g + i, 0]
            cwv[4 + 2 * i + 1] = conv_v[2 * g + i, 1]
        cw = np.broadcast_to(cwv, (128, 8)).astype(np.float32).copy()

        # stage0 blob: prefix + 8 groups of [wf(2k)|h0(2k)|wf(2k+1)|h0(2k+1)]
        st0 = np.empty((128, _ST0), bf)
        a = 0
        st0[:, a:a + 256] = cs[:, 0:256].astype(bf); a += 256
        st0[:, a:a + 256] = sn[:, 0:256].astype(bf); a += 256
        for k in range(NK):
            st0[:, a:a + 1024] = wpk[k * 128:(k + 1) * 128, :]; a += 1024
            st0[:, a:a + 256] = hT[k * 128:(k + 1) * 128, 0:256]; a += 256
        assert a == _ST0

        # chunks 1-7 blob: [t-1][k] -> hT[k*128:(k+1)*128, t*256:(t+1)*256]
        hb = np.empty((128, 7 * 4096), bf)
        for t in range(1, NT):
            for k in range(NK):
                hb[:, (t - 1) * 4096 + k * 256:(t - 1) * 4096 + (k + 1) * 256] = \
                    hT[k * 128:(k + 1) * 128, t * 256:(t + 1) * 256]

        wo = W_o[g * 512:(g + 1) * 512, :]
        wob = np.empty((128, QH * 2048), bf)
        for d in range(QH):
            wob[:, d * 2048:(d + 1) * 2048] = wo[d * 128:(d + 1) * 128, :]

        oe = np.concatenate([np.ones((128, 128), np.float32), eye], axis=1)

        in_maps.append({
            "cw": cw, "oe": oe, "st0": st0, "csn": csn, "hb": hb,
            "wob": wob, "msk": msk,
        })
    return in_maps


def run_cores(in_maps, trace=False, **kw):
    from concourse.bass_utils import run_bass_kernel_spmd
    if "nc" not in _CACHE:
        _CACHE["nc"] = _build_program()
    return run_bass_kernel_spmd(_CACHE["nc"], in_maps, list(range(NCORES)),
                                trace=trace, **kw)


def kernel(hidden, W_pack, W_o, conv_k, conv_v):
    hidden = np.asarray(hidden, np.float32)
    W_pack = np.asarray(W_pack, np.float32)
    W_o = np.asarray(W_o, np.float32)
    conv_k = np.asarray(conv_k, np.float32)
    conv_v = np.asarray(conv_v, np.float32)
    in_maps = _host_inputs(hidden, W_pack, W_o, conv_k, conv_v)
    res = run_cores(in_maps)
    out = np.zeros((B, S, H), np.float32)
    for c in range(NCORES):
        b = c // TP
        out[b] += res.results[c]["yT"].T
    return out
